# revision 5
# baseline (speedup 1.0000x reference)
"""Cross-attention (image<->text) kernel for TRN2, 8-core SPMD.

Problem: nn_CrossAttention. B=4, C=256, H=W=64 (Ni=4096), Lt=4096, Hd=128.

Sharding: 8 cores = 4 batches x 2 query-token halves. Each core computes
  att1 (img->text): queries = image tokens [half of 4096], keys/values = all text tokens
  att2 (text->img): queries = text tokens [half of 4096], keys/values = all image tokens
Outputs are disjoint slices -> no collectives; host gathers + transposes.

Per-core algorithm (fp32 PSUM accumulation throughout):
  - Host folds BN (eval) + the sqrt(Hd) score scale into the projection
    weights, pre-transposes them, and converts x to fp16: the kernel's
    prologue is DMA-only. V^T tiles (plus a ones column that yields the
    softmax denominator inside the AV accumulation) come host-pretransposed
    in bf16.
  - Projections (fp16) evict PSUM->SBUF via ACT twice: once fp16 (exact
    q/k) and once fp8e4m3 (stats copy); two small DMAs then lay the fp8
    copy out as [64, 2, N] k-tile pairs for DoubleRow matmuls.
  - Pass 1 (stats) runs entirely in fp8 DoubleRow (0.5 PE cycles/row):
    S8 = Q8 K8^T in [q, m] tiles; per-row max via DVE negated max-reduce.
    The resulting per-query shift b is only a ~+-30-accurate estimate of
    the true row max -- safe because P~ is bf16 (exp range e^+-88) and any
    consistent per-row shift cancels exactly in the final normalization.
  - Pass 2: exact fp16 scores S^T in [m, q] layout; the per-query -b shift
    is added with a single fp8 DoubleRow rank-1 matmul (ones x (-b/2) over
    2 k-tiles), half the PE cost of an fp16 rank-1 pair; exp on ACT ->
    bf16 P~; AV: out[q, C+1] += P~^T.T @ [V | 1] in bf16; normalization is
    one batched reciprocal + per-row scale at eviction.
  - Scheduling: the AV mc loop is software-pipelined (next chunk's score
    matmuls precede this chunk's AV) and the NEXT quarter's stats pairs are
    interleaved one-per-2-mc into the first mc slots, so the in-order PE
    queue never blocks on exp latency or on the stats reduces, and the bias
    row lands well before the next quarter starts.
"""

import numpy as np
import ml_dtypes

import concourse.bacc as bacc
import concourse.tile as tile
from concourse import mybir
from concourse.bass_utils import run_bass_kernel_spmd

F32 = mybir.dt.float32
F16 = mybir.dt.float16
BF16 = mybir.dt.bfloat16
F8 = mybir.dt.float8e4
AF = mybir.ActivationFunctionType
ALU = mybir.AluOpType
AX = mybir.AxisListType
DRM = mybir.MatmulPerfMode.DoubleRow

B, C, HD = 4, 256, 128
NI, LT = 4096, 4096
NQ = 2048        # query tokens per core (half)
M = 4096         # kv tokens (full)
SQ = float(128.0 ** 0.25)   # sqrt(sqrt(Hd)) folded into each of q and k
BN_EPS = 1e-5

N_CORES = 8


def _proj_tile(nc, pools, out_t, f8, wT, bias_ap, x, n0):
    """One 512-token projection tile: 2 matmuls (c halves) + fp16 ACT
    eviction + fp8 ACT eviction into tmp8, + 2 DMAs laying the fp8 copy out
    as [64, 2, 512] k-tile pairs (hd = t*64 + p) for DoubleRow stats.
    Uses the ps_p2 ring: projections never coexist with AV quarters, and this
    keeps them off the stats pairs' ps_p1 ring."""
    ps = pools["ps_p2"].tile([128, 512], F32, tag="ps_p2", name="ps_prj")
    for ch in range(2):
        nc.tensor.matmul(ps, wT[:, ch, :], x[:, ch, n0:n0 + 512],
                         start=(ch == 0), stop=(ch == 1))
    nc.scalar.activation(out_t[:, n0:n0 + 512], ps, AF.Identity,
                         bias=bias_ap, scale=1.0)
    tmp8, qk8 = f8
    nc.scalar.activation(tmp8[:, n0:n0 + 512], ps, AF.Identity,
                         bias=bias_ap, scale=1.0)
    nc.sync.dma_start(out=qk8[:, 0, n0:n0 + 512], in_=tmp8[0:64, n0:n0 + 512])
    nc.sync.dma_start(out=qk8[:, 1, n0:n0 + 512], in_=tmp8[64:128, n0:n0 + 512])


def _project(nc, pools, out_t, f8, wT, bias_ap, x, ntok):
    for n0 in range(0, ntok, 512):
        _proj_tile(nc, pools, out_t, f8, wT, bias_ap, x, n0)


def _stats_pair(nc, pools, st, qt, p):
    """One pass-1 unit: 2 fp8 DoubleRow score matmuls into one 2-bank psum
    tile + ONE negated max-reduce [128,2,512]->[128,2] on DVE."""
    q8, k8 = st["q8"], st["k8"]
    q_sl = q8[:, :, qt * 128:(qt + 1) * 128]
    ps = pools["ps_p1"].tile([128, 1024], F32, tag="ps_p1", name="ps_s",
                             bufs=1)
    for j in range(2):
        w = 2 * p + j
        nc.tensor.matmul(ps[:, j * 512:(j + 1) * 512], q_sl,
                         k8[:, :, w * 512:(w + 1) * 512],
                         start=True, stop=True, perf_mode=DRM)
    nc.vector.tensor_reduce(st["negm_all"][:, qt, 2 * p:2 * p + 2],
                            ps.rearrange("a (j w) -> a j w", w=512),
                            axis=AX.X, op=ALU.max, negate=True)


def _attn_stats_finalize_half(nc, pools, st, nq, h):
    """Combine pair maxes for qtile pair h of quarter nq -> fp8 -b/2 values,
    scattered into BOTH k-tiles of the quarter's bias row in linear-q order
    (so the DoubleRow rank-1 bias matmul reads a plain [1, 2, 512] AP)."""
    sp, tag = pools["stats"], st["tag"]
    sl = slice(nq * 4 + 2 * h, nq * 4 + 2 * h + 2)
    neg_mt = sp.tile([128, 2], F32, name=f"negM_{tag}_{nq}_{h}",
                     tag=f"negM_{tag}", bufs=2)
    nc.vector.tensor_reduce(neg_mt, st["negm_all"][:, sl, :], axis=AX.X,
                            op=ALU.min)
    # -b/4 per k-tile, summed by a twos-valued rank-1 DoubleRow matmul
    # (2*(-b/4)*2 ktiles = -b). /4 keeps magnitudes < ~170: the device
    # float8e4 is IEEE e4m3 (max finite 240, then inf) -- -b/2 would clamp
    # to -inf and zero out whole softmax rows.
    neg_b8 = sp.tile([128, 2], F8, tag=f"negb8_{tag}",
                     name=f"negb8_{tag}_{nq}_{h}", bufs=2)
    nc.vector.tensor_scalar_mul(neg_b8, neg_mt, 0.25)
    for t in range(2):
        for s in range(2):
            b0 = nq * 512 + h * 256 + s * 128
            nc.sync.dma_start(out=st["negb8"][0:1, t, b0:b0 + 128],
                              in_=neg_b8[:, s:s + 1])


def _stats_fillers(nc, pools, st, nq):
    """Emission closures for one quarter's stats: 16 pair units needing PE
    slots; the two finalizes ride along after pairs 7 and 15 (DVE/DMA only,
    no PE content)."""
    fs = []
    for k, (qt, p) in enumerate([(nq * 4 + i, p) for i in range(4)
                                 for p in range(4)]):
        def unit(qt=qt, p=p, k=k):
            _stats_pair(nc, pools, st, qt, p)
            if k == 7:
                _attn_stats_finalize_half(nc, pools, st, nq, 0)
            elif k == 15:
                _attn_stats_finalize_half(nc, pools, st, nq, 1)
        fs.append(unit)
    return fs


def _attn_av_quarter(nc, pools, st, nq, fillers=()):
    """Pass-2 S^T + DoubleRow rank-1 bias + exp + AV for one 512-query
    quarter.

    mc loop software-pipelined by one iteration (bias+score matmuls of chunk
    mc+1 are emitted BEFORE the AV matmuls of chunk mc); `fillers` (next
    quarter's stats) are emitted one per 2 mc slots."""
    qT, kT, v, tag = st["qT"], st["kT"], st["v"], st["tag"]
    ones8 = pools["ones8"]
    n_mc = M // 128
    v3 = v.rearrange("p (mc w) -> p mc w", w=257)
    n0 = nq * 512
    negb8_sl = st["negb8"][0:1, :, n0:n0 + 512]
    out_ps = [pools["ps_out"].tile([128, 257], F32, tag=f"out_ps{s}",
                                   name=f"out_ps{s}_{tag}_{nq}")
              for s in range(4)]

    def scores(mc):
        ps = pools["ps_p2"].tile([128, 512], F32, tag="ps_p2", name="ps_p2c")
        # the full-width score matmul OPENS the group (it depends only on
        # qT/kT); the DoubleRow bias closes it, so a late-arriving finalize
        # DMA never blocks the score matmul itself
        nc.tensor.matmul(ps, kT[:, mc * 128:(mc + 1) * 128],
                         qT[:, n0:n0 + 512], start=True, stop=False)
        nc.tensor.matmul(ps, ones8, negb8_sl, start=False, stop=True,
                         perf_mode=DRM, skip_group_check=True)
        pt = pools["pt"].tile([128, 512], BF16, tag="pt",
                              name=f"pt_{tag}_{nq}_{mc}")
        nc.scalar.activation(pt, ps, AF.Exp)
        return pt

    fillers = list(fillers)
    # one filler per 2 mc slots (hides the stats-reduce latency of the
    # single-buffer stats tile behind two mc of AV work)
    slot = {2 * k: k for k in range(len(fillers))}
    pt_cur = scores(0)
    for mc in range(n_mc):
        pt_nxt = scores(mc + 1) if mc + 1 < n_mc else None
        for s in range(4):
            nc.tensor.matmul(out_ps[s], pt_cur[:, s * 128:(s + 1) * 128],
                             v3[:, mc, :],
                             start=(mc == 0), stop=(mc == n_mc - 1))
        if mc in slot:
            fillers[slot[mc]]()
        pt_cur = pt_nxt
    for s in range(4):
        qt_idx = nq * 4 + s
        # stage unnormalized out + denominator on ACT (DVE's in-order queue
        # is full of next-quarter reduces; a late raw copy would WAR-block
        # the next quarter's first AV matmul on out_ps)
        nc.scalar.activation(st["raw"][:, qt_idx, :], out_ps[s], AF.Copy,
                             bias=0.0, scale=1.0)


def _attn_store_quarter(nc, pools, st, nq):
    """Reciprocal + normalize + DMA for one quarter."""
    sp, tag, out_dram = pools["stats"], st["tag"], st["out_dram"]
    raw = st["raw"]
    recip = sp.tile([128, 4], F32, tag=f"recip_{tag}",
                    name=f"recip_{tag}_{nq}", bufs=2)
    nc.vector.reciprocal(recip, raw[:, nq * 4:(nq + 1) * 4, 256])
    for s in range(4):
        qt = nq * 4 + s
        ostage = pools["ostage"].tile([128, 256], F32, tag="ostage",
                                      name=f"ostage_{tag}_{qt}")
        nc.vector.tensor_scalar_mul(ostage, raw[:, qt, 0:256],
                                    recip[:, s:s + 1])
        nc.sync.dma_start(out=out_dram[qt * 128:(qt + 1) * 128, :], in_=ostage)


def _attn_state(nc, pools, tag, qT, kT, q8, k8, v, out_dram):
    sp = pools["stats"]
    n_qt = NQ // 128
    st = {"tag": tag, "qT": qT, "kT": kT, "q8": q8, "k8": k8, "v": v,
          "out_dram": out_dram}
    st["negb8"] = sp.tile([1, 2, NQ], F8, name=f"negb8row_{tag}",
                          tag=f"negb8row_{tag}", bufs=1)
    st["negm_all"] = sp.tile([128, n_qt, 8], F32, name=f"negmall_{tag}",
                             tag=f"negmall_{tag}", bufs=1)
    st["raw"] = sp.tile([128, n_qt, 257], F32, name=f"raw_{tag}",
                        tag=f"raw_{tag}", bufs=1)
    return st


def build_nc(reps=1):
    """Build the SPMD Bass program (identical for all cores).

    Query tokens are always slice [0:NQ] of the token axis. The host feeds
    half-1 cores an x whose two token halves are swapped: queries then sit in
    [0:NQ], while keys/values see a permuted-but-consistent full token set
    (softmax+AV are invariant to a joint permutation of keys and values)."""
    nc = bacc.Bacc(None)

    x_i = nc.dram_tensor("x_i", [2, 128, NI], F16, kind="ExternalInput")
    x_t = nc.dram_tensor("x_t", [2, 128, LT], F16, kind="ExternalInput")
    # host-pretransposed V (+ ones column): [m-in-chunk, mc, C+1]
    v_td = nc.dram_tensor("v_td", [128, M // 128, 257], BF16, kind="ExternalInput")
    v_id = nc.dram_tensor("v_id", [128, M // 128, 257], BF16, kind="ExternalInput")
    # host-folded, pre-transposed fp16 weights [c-in-half, ch, hd]
    wq = nc.dram_tensor("wq", [128, 2, 128], F16, kind="ExternalInput")
    wk = nc.dram_tensor("wk", [128, 2, 128], F16, kind="ExternalInput")
    wtq = nc.dram_tensor("wtq", [128, 2, 128], F16, kind="ExternalInput")
    wtk = nc.dram_tensor("wtk", [128, 2, 128], F16, kind="ExternalInput")
    # host-folded projection biases [128, 4]: i_q, i_k, t_k, t_q
    biases = nc.dram_tensor("biases", [128, 4], F32, kind="ExternalInput")
    out_i = nc.dram_tensor("out_i", [NQ, C], F32, kind="ExternalOutput")
    out_t = nc.dram_tensor("out_t", [NQ, C], F32, kind="ExternalOutput")

    with tile.TileContext(nc) as tc:
        import contextlib
        with contextlib.ExitStack() as ctx:

            pools = {}
            pools["consts"] = ctx.enter_context(tc.tile_pool(name="consts", bufs=1))
            pools["stats"] = ctx.enter_context(tc.tile_pool(name="stats", bufs=2))
            pools["xpool"] = ctx.enter_context(tc.tile_pool(name="xpool", bufs=1))
            pools["vpool"] = ctx.enter_context(tc.tile_pool(name="vpool", bufs=1))
            pools["qkpool"] = ctx.enter_context(tc.tile_pool(name="qkpool", bufs=1))
            pools["f8pool"] = ctx.enter_context(tc.tile_pool(name="f8pool", bufs=1))
            pools["pt"] = ctx.enter_context(tc.tile_pool(name="pt", bufs=5))
            pools["ostage"] = ctx.enter_context(tc.tile_pool(name="ostage", bufs=4))
            pools["ps_p1"] = ctx.enter_context(
                tc.tile_pool(name="ps_p1", bufs=2, space="PSUM"))
            pools["ps_p2"] = ctx.enter_context(
                tc.tile_pool(name="ps_p2", bufs=2, space="PSUM"))
            pools["ps_out"] = ctx.enter_context(
                tc.tile_pool(name="ps_out", bufs=1, space="PSUM"))

            consts = pools["consts"]

            # ---- weight/bias tiles (DMAs emitted after the first xt chunk
            # below: transfers serialize on the DMA engines, and t_kT needs
            # xt c0 + wtk first) ----
            wT = {}
            for name in ("wq", "wk", "wtq", "wtk"):
                wT[name] = consts.tile([128, 2, 128], F16, name=f"wT_{name}")
            bias_t = consts.tile([128, 4], F32, name="bias_t")

            twos_f = consts.tile([1, 256], F32, name="twos_row_f")
            nc.vector.memset(twos_f, 2.0)
            ones8_t = consts.tile([1, 2, 128], F8, name="ones8_row")
            nc.vector.tensor_copy(ones8_t.rearrange("o t p -> o (t p)"), twos_f)
            pools["ones8"] = ones8_t

            for _rep in range(reps):
                qk = pools["qkpool"]
                t_kT = qk.tile([128, LT], F16, name="t_kT")
                t_qT = qk.tile([128, NQ], F16, name="t_qT")
                i_qT = qk.tile([128, NQ], F16, name="i_qT")
                i_kT = qk.tile([128, NI], F16, name="i_kT")
                f8p = pools["f8pool"]
                t_k8 = f8p.tile([64, 2, LT], F8, name="t_k8")
                t_q8 = f8p.tile([64, 2, NQ], F8, name="t_q8")
                i_q8 = f8p.tile([64, 2, NQ], F8, name="i_q8")
                i_k8 = f8p.tile([64, 2, NI], F8, name="i_k8")
                tmp8 = {"t_k": f8p.tile([128, LT], F8, name="tmp8_tk"),
                        "t_q": f8p.tile([128, NQ], F8, name="tmp8_tq"),
                        "i_q": f8p.tile([128, NQ], F8, name="tmp8_iq"),
                        "i_k": f8p.tile([128, NI], F8, name="tmp8_ik")}
                v_t = pools["vpool"].tile([128, (M // 128) * 257], BF16, name="v_t")
                v_i = pools["vpool"].tile([128, (M // 128) * 257], BF16, name="v_i")
                v3_t = v_t.rearrange("p (mc w) -> p mc w", w=257)
                v3_i = v_i.rearrange("p (mc w) -> p mc w", w=257)

                # ---- input loads: fp16 x straight into SBUF; xt on the SP
                # hwdge queue, xi on the ACT queue ----
                xt = pools["xpool"].tile([128, 2, LT], F16, tag="x_t", name="xt")
                xi = pools["xpool"].tile([128, 2, NI], F16, tag="x_i", name="xi")
                HC = 2048

                def xdma(eng, dst, src, c, ch):
                    eng.dma_start(out=dst[:, ch, c * HC:(c + 1) * HC],
                                  in_=src[ch, :, c * HC:(c + 1) * HC])

                # SP queue: xt c0 pair + weights lead (t_kT gates the
                # whole pipeline); ACT queue: xi (i_qT + first stats cell)
                xdma(nc.sync, xt, x_t, 0, 0)
                xdma(nc.sync, xt, x_t, 0, 1)
                for name, dram in (("wtk", wtk), ("wq", wq), ("wk", wk),
                                   ("wtq", wtq)):
                    nc.sync.dma_start(out=wT[name], in_=dram[:, :, :])
                nc.sync.dma_start(out=bias_t, in_=biases[:, :])
                for c in range(NI // HC):
                    for ch in range(2):
                        xdma(nc.scalar, xi, x_i, c, ch)
                xdma(nc.sync, xt, x_t, 1, 0)
                xdma(nc.sync, xt, x_t, 1, 1)

                # ---- V^T comes host-pretransposed (with ones column) ----
                nc.sync.dma_start(out=v3_t, in_=v_td[:, :, :])
                nc.sync.dma_start(out=v3_i, in_=v_id[:, :, :])

                # ---- projections needed by the first stats cell ----
                _project(nc, pools, t_kT, (tmp8["t_k"], t_k8), wT["wtk"],
                         bias_t[:, 2:3], xt, LT)
                _project(nc, pools, i_qT, (tmp8["i_q"], i_q8), wT["wq"],
                         bias_t[:, 0:1], xi, NQ)

                a1 = _attn_state(nc, pools, "a1", i_qT, t_kT, i_q8, t_k8,
                                 v_t, out_i)
                a2 = _attn_state(nc, pools, "a2", t_qT, i_kT, t_q8, i_k8,
                                 v_i, out_t)

                # ---- prologue: cell-0 stats interleaved with the remaining
                # projections (t_qT, i_kT) so PE stays fed while DVE reduces
                fill0 = _stats_fillers(nc, pools, a1, 0)
                units = [lambda n0=n0: _proj_tile(nc, pools, t_qT,
                                                  (tmp8["t_q"], t_q8),
                                                  wT["wtq"], bias_t[:, 3:4],
                                                  xt, n0)
                         for n0 in range(0, NQ, 512)]
                units += [lambda n0=n0: _proj_tile(nc, pools, i_kT,
                                                   (tmp8["i_k"], i_k8),
                                                   wT["wk"], bias_t[:, 1:2],
                                                   xi, n0)
                          for n0 in range(0, NI, 512)]
                for k in range(len(fill0)):
                    fill0[k]()
                    if k < len(units):
                        units[k]()
                for k in range(len(fill0), len(units)):
                    units[k]()

                cells = [(a1, q) for q in range(NQ // 512)] + \
                        [(a2, q) for q in range(NQ // 512)]
                for i, (att, q) in enumerate(cells):
                    if i + 1 < len(cells):
                        nxt = _stats_fillers(nc, pools, cells[i + 1][0],
                                             cells[i + 1][1])
                    else:
                        nxt = []
                    _attn_av_quarter(nc, pools, att, q, nxt)
                    _attn_store_quarter(nc, pools, att, q)

    nc.compile()
    return nc


_NC_CACHE = {}


def _get_nc():
    if "nc" not in _NC_CACHE:
        _NC_CACHE["nc"] = build_nc()
    return _NC_CACHE["nc"]


def _swapped(x_flat, h):
    """x_flat [C, Ntok] fp16, token halves swapped when h==1."""
    x_flat = x_flat.astype(np.float16)
    if h:
        n = x_flat.shape[1]
        x_flat = np.concatenate([x_flat[:, n // 2:], x_flat[:, :n // 2]], axis=1)
    return x_flat


def _v_pack(xs):
    """xs [C, M] -> [128, M/128, 257] x^T chunks + ones column (bf16)."""
    v = np.empty((128, M // 128, 257), ml_dtypes.bfloat16)
    v[:, :, 0:256] = xs.astype(np.float32).T.reshape(
        M // 128, 128, C).transpose(1, 0, 2).astype(ml_dtypes.bfloat16)
    v[:, :, 256] = 1.0
    return np.ascontiguousarray(v)


def _fold_weight(w, scale):
    """w [Hd, C] * per-row scale -> pre-transposed fp16 [c-in-half, ch, hd]."""
    wp = (w * scale[:, None]).astype(np.float32)
    arr = wp.T.reshape(2, 128, 128).transpose(1, 0, 2)
    return np.ascontiguousarray(arr.astype(np.float16))


def run_spmd(inputs, **kw):
    """Build in_maps, run on 8 cores, return BassKernelResults."""
    nc = _get_nc()
    s_q = inputs["bnq_gamma"] / np.sqrt(inputs["bnq_var"] + BN_EPS)
    s_k = inputs["bnk_gamma"] / np.sqrt(inputs["bnk_var"] + BN_EPS)
    wq = _fold_weight(inputs["w_img_q"], s_q * SQ)
    wk = _fold_weight(inputs["w_img_k"], s_k * SQ)
    wtq = _fold_weight(inputs["w_text_q"], np.full(HD, SQ, np.float32))
    wtk = _fold_weight(inputs["w_text_k"], np.full(HD, SQ, np.float32))
    biases = np.ascontiguousarray(np.stack([
        (inputs["bnq_beta"] - inputs["bnq_mean"] * s_q) * SQ,
        (inputs["bnk_beta"] - inputs["bnk_mean"] * s_k) * SQ,
        inputs["b_text_k"] * SQ,
        inputs["b_text_q"] * SQ,
    ], axis=1).astype(np.float32))
    in_maps = []
    for core in range(N_CORES):
        b, h = core // 2, core % 2
        xsi = _swapped(inputs["input_i"][b].reshape(C, NI), h)
        xst = _swapped(inputs["input_t"][b].reshape(C, LT), h)
        m = {
            "x_i": np.ascontiguousarray(xsi.reshape(2, 128, NI)),
            "x_t": np.ascontiguousarray(xst.reshape(2, 128, LT)),
            "v_td": _v_pack(xst), "v_id": _v_pack(xsi),
            "wq": wq, "wk": wk, "wtq": wtq, "wtk": wtk,
            "biases": biases,
        }
        in_maps.append(m)
    res = run_bass_kernel_spmd(nc, in_maps, list(range(N_CORES)), **kw)
    return res


def gather(res):
    output_i = np.empty((B, NI, C), np.float32)
    output_t = np.empty((B, LT, C), np.float32)
    for core in range(N_CORES):
        b, h = core // 2, core % 2
        r = res.results[core]
        output_i[b, h * NQ:(h + 1) * NQ, :] = np.asarray(r["out_i"])
        output_t[b, h * NQ:(h + 1) * NQ, :] = np.asarray(r["out_t"])
    return (output_i, output_t)


def kernel(**inputs):
    inputs = {k: np.asarray(v, dtype=np.float32) for k, v in inputs.items()}
    res = run_spmd(inputs)
    return gather(res)
